# revision 23
# baseline (speedup 1.0000x reference)
# kernel.py — MultiHeadAttention (B=1, S=L=4096, D=1024, H=16, hd=64)
# 8-core head-tensor-parallel Bass kernel for Trainium2.
#
# Sharding: 2 heads per core. Each core receives transposed activations
# qT/kT/vT (D, S) plus its per-core weight slices, computes its two heads'
# attention and the row-sharded output projection, and returns a partial
# (S, D) output. The host sums the 8 partials and adds bo.
import numpy as np

DIM = 1024
NHEAD = 16
HEAD_DIM = 64
S = 4096
L = 4096
NCORES = 8
HPC = NHEAD // NCORES          # heads per core = 2
CSLICE = HPC * HEAD_DIM        # per-core feature slice = 128

_CACHE = {}


def _build_nc(io_dt_name="float32r", upto=99):
    """Build the per-core Bass program (same program on all 8 cores)."""
    import contextlib

    import concourse.bass as bass
    import concourse.mybir as mybir
    import concourse.tile as tile
    from concourse import bacc
    from concourse.masks import make_identity

    io_dt = getattr(mybir.dt, io_dt_name)
    f32 = mybir.dt.float32

    def mm(ap):
        return ap

    nc = bacc.Bacc("TRN2", target_bir_lowering=False)

    # DRAM tensors (per core)
    qT = nc.dram_tensor("qT", [DIM, S], io_dt, kind="ExternalInput")
    kT = nc.dram_tensor("kT", [DIM, L], io_dt, kind="ExternalInput")
    vT = nc.dram_tensor("vT", [DIM, L], io_dt, kind="ExternalInput")
    wqT = nc.dram_tensor("wqT", [DIM, CSLICE], io_dt, kind="ExternalInput")
    wkT = nc.dram_tensor("wkT", [DIM, CSLICE], io_dt, kind="ExternalInput")
    wvT = nc.dram_tensor("wvT", [DIM, CSLICE], io_dt, kind="ExternalInput")
    bqc = nc.dram_tensor("bqc", [CSLICE, 1], f32, kind="ExternalInput")
    bkc = nc.dram_tensor("bkc", [CSLICE, 1], f32, kind="ExternalInput")
    bvc = nc.dram_tensor("bvc", [CSLICE, 1], f32, kind="ExternalInput")
    woS = nc.dram_tensor("woS", [CSLICE, DIM], io_dt, kind="ExternalInput")
    out = nc.dram_tensor("out", [S, DIM], f32, kind="ExternalOutput")

    NC_CHUNKS = DIM // 128      # 8 contraction chunks for projections
    NSQ = S // 512              # 8 query tiles of width 512
    NSK = L // 128              # 32 key/value chunks of 128
    VW = 2 * (HEAD_DIM + 1)     # 130: [v_h0 | 1 | v_h1 | 1]

    with tile.TileContext(nc) as tc:
        with contextlib.ExitStack() as ctx:
            singles = ctx.enter_context(tc.tile_pool(name="singles", bufs=1))
            persist = ctx.enter_context(tc.tile_pool(name="persist", bufs=1))
            stream = ctx.enter_context(tc.tile_pool(name="stream", bufs=2))
            ps = ctx.enter_context(
                tc.tile_pool(name="ps", bufs=2, space="PSUM"))
            probs_pool = ctx.enter_context(tc.tile_pool(name="probs", bufs=3))
            small = ctx.enter_context(tc.tile_pool(name="small", bufs=2))
            outb_pool = ctx.enter_context(tc.tile_pool(name="outb", bufs=3))

            # ---- constants ----
            w_q = singles.tile([128, NC_CHUNKS, 128], io_dt, tag="wq")
            w_k = singles.tile([128, NC_CHUNKS, 128], io_dt, tag="wk")
            w_v = singles.tile([128, NC_CHUNKS, 128], io_dt, tag="wv")
            nc.sync.dma_start(w_q[:], wqT.rearrange("(c p) m -> p c m", p=128))
            nc.sync.dma_start(w_k[:], wkT.rearrange("(c p) m -> p c m", p=128))
            nc.sync.dma_start(w_v[:], wvT.rearrange("(c p) m -> p c m", p=128))
            b_q = singles.tile([128, 1], f32, tag="bq")
            b_k = singles.tile([128, 1], f32, tag="bk")
            b_v = singles.tile([128, 1], f32, tag="bv")
            nc.sync.dma_start(b_q[:], bqc[:, :])
            nc.sync.dma_start(b_k[:], bkc[:, :])
            nc.sync.dma_start(b_v[:], bvc[:, :])
            wo_sb = singles.tile([128, DIM], io_dt, tag="wo")
            nc.sync.dma_start(wo_sb[:], woS[:, :])
            # h1 slice of woS re-based to partitions 0..63 for K=64 matmuls
            wo1_sb = singles.tile([64, DIM], io_dt, tag="wo1")
            nc.sync.dma_start(wo1_sb[:], woS[64:128, :])
            ident = singles.tile([128, 128], f32, tag="ident")
            make_identity(nc, ident[:])
            ones64 = singles.tile([1, 64], io_dt, tag="ones64")
            nc.vector.memset(ones64[:].bitcast(mybir.dt.uint32), 0x3F800000)

            # ---- persistent activations ----
            qt_sb = persist.tile([128, S], io_dt, tag="qt")
            kt_sb = persist.tile([128, L], io_dt, tag="kt")
            vt_sb = persist.tile([128, L], f32, tag="vt")
            v_sb = persist.tile([128, NSK, VW], io_dt, tag="v")
            ctxu0 = persist.tile([64, S], io_dt, tag="ctxu0")
            ctxu1 = persist.tile([64, S], io_dt, tag="ctxu1")
            zq = persist.tile([128, 64], f32, tag="zq")

            # ---- phase A/B/C: projections (transposed layouts) ----
            for (src, w_t, b_t, dst) in (
                (qT, w_q, b_q, qt_sb),
                (kT, w_k, b_k, kt_sb),
                (vT, w_v, b_v, vt_sb),
            ):
                for n in range(NSQ):
                    ps_t = ps.tile([128, 512], f32, tag="sc")
                    for c in range(NC_CHUNKS):
                        xt = stream.tile([128, 512], io_dt, tag="xin")
                        nc.sync.dma_start(
                            xt[:], src[c * 128:(c + 1) * 128,
                                       n * 512:(n + 1) * 512])
                        nc.tensor.matmul(
                            ps_t[:], mm(w_t[:, c, :]), mm(xt[:]),
                            start=(c == 0), stop=(c == NC_CHUNKS - 1))
                    nc.vector.tensor_scalar_add(
                        dst[:, n * 512:(n + 1) * 512], ps_t[:], b_t[:])

            if upto >= 2:
                # ---- phase C2: transpose VT -> V, add ones columns ----
                # layout per chunk: [v_h0 (64) | 1 | v_h1 (64) | 1]
                nc.vector.memset(
                    v_sb[:, :, HEAD_DIM:HEAD_DIM + 1].bitcast(mybir.dt.uint32),
                    0x3F800000)
                nc.vector.memset(
                    v_sb[:, :, VW - 1:VW].bitcast(mybir.dt.uint32),
                    0x3F800000)
                for t in range(NSK):
                    pt = ps.tile([128, 128], f32, tag="cx0")
                    nc.tensor.transpose(
                        pt[:], vt_sb[:, t * 128:(t + 1) * 128], ident[:])
                    nc.vector.tensor_copy(v_sb[:, t, 0:HEAD_DIM],
                                          pt[:, 0:HEAD_DIM])
                    nc.vector.tensor_copy(v_sb[:, t, HEAD_DIM + 1:VW - 1],
                                          pt[:, HEAD_DIM:128])

            if upto >= 3:
                # ---- phase D: attention (both heads concurrently) ----
                for sq in range(NSQ):
                    cx0 = ps.tile([128, 512], f32, tag="cx0")
                    cx1 = ps.tile([128, 512], f32, tag="cx1")
                    for c in range(NSK):
                        sc = ps.tile([128, 1024], f32, tag="sc")
                        # scoresT = K_h[:, chunk].T @ Q_h per head (row groups)
                        nc.tensor.matmul(
                            sc[:, 0:512],
                            mm(kt_sb[0:64, c * 128:(c + 1) * 128]),
                            mm(qt_sb[0:64, sq * 512:(sq + 1) * 512]),
                            start=True, stop=True)
                        nc.tensor.matmul(
                            sc[:, 512:1024],
                            mm(kt_sb[64:128, c * 128:(c + 1) * 128]),
                            mm(qt_sb[64:128, sq * 512:(sq + 1) * 512]),
                            start=True, stop=True)
                        pr = probs_pool.tile([128, 1024], io_dt, tag="pr")
                        nc.scalar.activation(
                            pr[:], sc[:], mybir.ActivationFunctionType.Exp,
                            scale=float(1.0 / np.sqrt(HEAD_DIM)))
                        # ctx_h += [v_h | 1].T @ probsT_h  (row 64 = Z)
                        nc.tensor.matmul(
                            cx0[0:65, :], mm(v_sb[:, c, 0:HEAD_DIM + 1]),
                            mm(pr[:, 0:512]),
                            start=(c == 0), stop=(c == NSK - 1))
                        nc.tensor.matmul(
                            cx1[0:65, :], mm(v_sb[:, c, HEAD_DIM + 1:VW]),
                            mm(pr[:, 512:1024]),
                            start=(c == 0), stop=(c == NSK - 1))
                    # evacuate ctx (unnormalized) and Z rows
                    sl = slice(sq * 512, (sq + 1) * 512)
                    nc.vector.tensor_copy(ctxu0[0:64, sl], cx0[0:64, :])
                    nc.vector.tensor_copy(ctxu1[0:64, sl], cx1[0:64, :])
                    # stage Z rows then scatter into lane-parallel zq:
                    # zq[p, h*32 + t] = Z_h[p*32 + t]
                    zr0 = small.tile([65, 512], f32, tag="zr0")
                    zr1 = small.tile([65, 512], f32, tag="zr1")
                    nc.vector.tensor_copy(zr0[64:65, :], cx0[64:65, :])
                    nc.vector.tensor_copy(zr1[64:65, :], cx1[64:65, :])
                    psl = slice(sq * 16, (sq + 1) * 16)
                    nc.sync.dma_start(zq[psl, 0:32], zr0[64:65, :])
                    nc.sync.dma_start(zq[psl, 32:64], zr1[64:65, :])

            if upto >= 4:
                # ---- phase E: normalize + output projection ----
                rz = small.tile([128, 64], io_dt, tag="rz")
                with nc.allow_low_precision(
                        reason="1/Z at f32r precision is ample"):
                    nc.vector.reciprocal(rz[:], zq[:])
                # back to rows: rzrow[h][0, i] = 1/Z_h[i]  (i = p*32 + t)
                rzrow0 = singles.tile([1, S], io_dt, tag="rzrow0")
                rzrow1 = singles.tile([1, S], io_dt, tag="rzrow1")
                nc.sync.dma_start(rzrow0[:, :], rz[:, 0:32])
                nc.sync.dma_start(rzrow1[:, :], rz[:, 32:64])
                for sq in range(NSQ):
                    sl = slice(sq * 512, (sq + 1) * 512)
                    # broadcast 1/Z along partitions via K=1 outer product
                    rzb0 = ps.tile([64, 512], f32, tag="cx0")
                    rzb1 = ps.tile([64, 512], f32, tag="cx1")
                    nc.tensor.matmul(rzb0[:], mm(ones64[:]),
                                     mm(rzrow0[:, sl]), start=True, stop=True)
                    nc.tensor.matmul(rzb1[:], mm(ones64[:]),
                                     mm(rzrow1[:, sl]), start=True, stop=True)
                    nc.vector.tensor_mul(ctxu0[:, sl], ctxu0[:, sl], rzb0[:])
                    nc.vector.tensor_mul(ctxu1[:, sl], ctxu1[:, sl], rzb1[:])
                for t in range(S // 128):
                    po = ps.tile([128, 1024], f32, tag="sc")
                    for od in range(2):
                        nc.tensor.matmul(
                            po[:, od * 512:(od + 1) * 512],
                            mm(ctxu0[:, t * 128:(t + 1) * 128]),
                            mm(wo_sb[0:64, od * 512:(od + 1) * 512]),
                            start=True, stop=False)
                        nc.tensor.matmul(
                            po[:, od * 512:(od + 1) * 512],
                            mm(ctxu1[:, t * 128:(t + 1) * 128]),
                            mm(wo1_sb[:, od * 512:(od + 1) * 512]),
                            start=False, stop=True)
                    ob = outb_pool.tile([128, 1024], f32, tag="ob")
                    nc.vector.tensor_copy(ob[:], po[:])
                    nc.sync.dma_start(out[t * 128:(t + 1) * 128, :], ob[:])
            else:
                # staged debug output so every build writes `out`
                if upto >= 3:
                    nc.sync.dma_start(out[0:64, :],
                                      ctxu0[:, 0:1024].bitcast(f32))
                elif upto >= 2:
                    nc.sync.dma_start(
                        out[0:128, :],
                        v_sb[:, 0:8, 0:128].rearrange(
                            "p a b -> p (a b)").bitcast(f32))
                else:
                    nc.sync.dma_start(out[0:128, :],
                                      qt_sb[:, 0:1024].bitcast(f32))

    nc.compile()
    return nc


def _get_nc(io_dt_name="float32r", upto=99):
    key = (io_dt_name, upto)
    if key not in _CACHE:
        _CACHE[key] = _build_nc(io_dt_name, upto)
    return _CACHE[key]


def _prepare_in_maps(inputs, io_np=np.float32):
    query, key, value = inputs["query"], inputs["key"], inputs["value"]
    Wq, Wk, Wv, Wo = inputs["Wq"], inputs["Wk"], inputs["Wv"], inputs["Wo"]
    bq, bk, bv = inputs["bq"], inputs["bk"], inputs["bv"]

    q2 = np.ascontiguousarray(query.reshape(S, DIM).T).astype(io_np)
    k2 = np.ascontiguousarray(key.reshape(L, DIM).T).astype(io_np)
    v2 = np.ascontiguousarray(value.reshape(L, DIM).T).astype(io_np)

    in_maps = []
    for c in range(NCORES):
        sl = slice(c * CSLICE, (c + 1) * CSLICE)
        in_maps.append({
            "qT": q2, "kT": k2, "vT": v2,
            "wqT": np.ascontiguousarray(Wq[sl, :].T).astype(io_np),
            "wkT": np.ascontiguousarray(Wk[sl, :].T).astype(io_np),
            "wvT": np.ascontiguousarray(Wv[sl, :].T).astype(io_np),
            "bqc": np.ascontiguousarray(bq[sl].reshape(CSLICE, 1)).astype(np.float32),
            "bkc": np.ascontiguousarray(bk[sl].reshape(CSLICE, 1)).astype(np.float32),
            "bvc": np.ascontiguousarray(bv[sl].reshape(CSLICE, 1)).astype(np.float32),
            "woS": np.ascontiguousarray(Wo[:, sl].T).astype(io_np),
        })
    return in_maps


def kernel(query, key, value, Wq, bq, Wk, bk, Wv, bv, Wo, bo):
    from concourse.bass_utils import run_bass_kernel_spmd

    nc = _get_nc()
    in_maps = _prepare_in_maps(dict(
        query=query, key=key, value=value, Wq=Wq, Wk=Wk, Wv=Wv, Wo=Wo,
        bq=bq, bk=bk, bv=bv))

    res = run_bass_kernel_spmd(nc, in_maps, core_ids=list(range(NCORES)))
    acc = np.zeros((S, DIM), np.float64)
    for r in res.results:
        acc += r["out"].astype(np.float64)
    acc += bo.astype(np.float64)
    return acc.astype(np.float32).reshape(1, S, DIM)
